# revision 8
# baseline (speedup 1.0000x reference)
"""Multi-head causal attention (B=2, T=1024, C=1024, H=16) on 8 TRN2 NeuronCores.

Sharding: head/batch parallel. Core c owns heads {2c, 2c+1} for both batches
(4 (b,h) pairs per core). Q/K/V [B,T,C] are host-transposed to [C, B*T] so
every device matmul operand has its contraction dim on SBUF partitions —
no on-device transposes anywhere.

Per-core device pipeline (all fp32):
  1. qT,kT = W @ X^T  -> [128(2 heads x 64), 1024] per batch  (transposed proj)
     v     = X @ W^T  -> [128(s), 128(2 heads x 64)] tiles    (natural proj)
  2. ST = k @ q^T per (b,h): scores in TRANSPOSED layout [s, t]; causal
     lower-triangular tiles only; additive mask on diagonal tiles; exp on ACT
     with the 1/8 scale folded in.
  3. A@V with v augmented by a ones column: one matmul chain yields both
     yT_unnorm = v^T @ exp(ST) and the softmax denominators (row 64).
  4. Normalize: reciprocal of denominators, partition-broadcast via DMA,
     scale yT and the exp(ST) tiles (-> attention probs, stored transposed).
  5. y_partial = yT^T @ Wp^T-slice; host sums the 8 partials and adds bp.

Host assembles: a[b, 2c+hl] = a_core[c][2b+hl]^T, y = sum(y_partial) + bp.
"""

import numpy as np

B, T, C, H = 2, 1024, 1024, 16
HD = C // H          # 64
NCORES = 8
HPC = H // NCORES    # 2 heads per core
PD = HPC * HD        # 128 projected dims per core
NP = B * HPC         # 4 (b,h) pairs per core
NI = T // 128        # 8 s-tiles of 128
NJ = T // 512        # 2 t-chunks of 512

_CACHE = {}


def _build_program():
    import concourse.bass as bass
    import concourse.tile as tile
    import concourse.mybir as mybir
    from concourse import bacc

    f32 = mybir.dt.float32
    AF = mybir.ActivationFunctionType

    nc = bacc.Bacc("TRN2", target_bir_lowering=False, debug=False, num_devices=NCORES)

    qt_d = nc.dram_tensor("qt", [C, B * T], f32, kind="ExternalInput")
    kt_d = nc.dram_tensor("kt", [C, B * T], f32, kind="ExternalInput")
    vt_d = nc.dram_tensor("vt", [C, B * T], f32, kind="ExternalInput")
    wqt_d = nc.dram_tensor("wqt", [128, 8, PD], f32, kind="ExternalInput")
    wkt_d = nc.dram_tensor("wkt", [128, 8, PD], f32, kind="ExternalInput")
    wvt_d = nc.dram_tensor("wvt", [128, 8, PD], f32, kind="ExternalInput")
    wpt_d = nc.dram_tensor("wpt", [PD, C], f32, kind="ExternalInput")
    bq_d = nc.dram_tensor("bq", [PD, 1], f32, kind="ExternalInput")
    bk_d = nc.dram_tensor("bk", [PD, 1], f32, kind="ExternalInput")
    bv_d = nc.dram_tensor("bv", [PD], f32, kind="ExternalInput")
    ma_d = nc.dram_tensor("maskadd", [128, NI, 512], f32, kind="ExternalInput")
    a_out = nc.dram_tensor("a_out", [NP, T, T], f32, kind="ExternalOutput")
    y_out = nc.dram_tensor("y_out", [B * T, C], f32, kind="ExternalOutput")

    from contextlib import ExitStack

    with tile.TileContext(nc) as tc, ExitStack() as ctx:
        consts = ctx.enter_context(tc.tile_pool(name="consts", bufs=1))
        instream = ctx.enter_context(tc.tile_pool(name="instream", bufs=4))
        qkres = ctx.enter_context(tc.tile_pool(name="qkres", bufs=1))
        vres = ctx.enter_context(tc.tile_pool(name="vres", bufs=1))
        expp = ctx.enter_context(tc.tile_pool(name="expp", bufs=10))
        rrow_p = ctx.enter_context(tc.tile_pool(name="rrow", bufs=4))
        rbc_p = ctx.enter_context(tc.tile_pool(name="rbc", bufs=4))
        rdram = ctx.enter_context(tc.tile_pool(name="rdram", bufs=4, space="DRAM"))
        ytp = ctx.enter_context(tc.tile_pool(name="ytp", bufs=1))
        youtp = ctx.enter_context(tc.tile_pool(name="youtp", bufs=3))
        pp = ctx.enter_context(tc.tile_pool(name="pp", bufs=4, space="PSUM"))

        # ---- resident constants ----
        wq_sb = consts.tile([128, 8, PD], f32)
        nc.sync.dma_start(out=wq_sb, in_=wqt_d[:, :, :])
        wk_sb = consts.tile([128, 8, PD], f32)
        nc.sync.dma_start(out=wk_sb, in_=wkt_d[:, :, :])
        wv_sb = consts.tile([128, 8, PD], f32)
        nc.sync.dma_start(out=wv_sb, in_=wvt_d[:, :, :])
        wp_sb = consts.tile([PD, C], f32)
        nc.sync.dma_start(out=wp_sb, in_=wpt_d[:, :])
        ma_sb = consts.tile([128, NI, 512], f32)
        nc.sync.dma_start(out=ma_sb, in_=ma_d[:, :, :])
        bq_sb = consts.tile([PD, 1], f32)
        nc.sync.dma_start(out=bq_sb, in_=bq_d[:, :])
        bk_sb = consts.tile([PD, 1], f32)
        nc.sync.dma_start(out=bk_sb, in_=bk_d[:, :])
        # bv broadcast to all partitions: [128, PD]
        bv_sb = consts.tile([128, PD], f32)
        bv_bcast = bass.AP(tensor=bv_d, offset=0, ap=[[0, 128], [1, PD]])
        nc.gpsimd.dma_start(out=bv_sb, in_=bv_bcast)

        # ---- q/k transposed projections: out[b] = [128(2h*64), 1024(t)] ----
        qt_sb = [qkres.tile([PD, T], f32, tag=f"qt{b}", name=f"qt_sb{b}") for b in range(B)]
        kt_sb = [qkres.tile([PD, T], f32, tag=f"kt{b}", name=f"kt_sb{b}") for b in range(B)]
        for name, x_d, w_sb, b_sb, dst in (
            ("q", qt_d, wq_sb, bq_sb, qt_sb),
            ("k", kt_d, wk_sb, bk_sb, kt_sb),
        ):
            ps = [pp.tile([PD, T], f32, tag="ps_proj", bufs=2, name="ps_proj") for _ in range(B)]
            for ci in range(8):
                x_t = instream.tile([128, B * T], f32, tag="xin", bufs=8, name="x_t")
                nc.sync.dma_start(out=x_t, in_=x_d[ci * 128 : (ci + 1) * 128, :])
                for b in range(B):
                    for j in range(NJ):
                        nc.tensor.matmul(
                            ps[b][:, j * 512 : (j + 1) * 512],
                            lhsT=w_sb[:, ci, :],
                            rhs=x_t[:, b * T + j * 512 : b * T + (j + 1) * 512],
                            start=(ci == 0),
                            stop=(ci == 7),
                        )
            for b in range(B):
                # evict + bias (bias varies along partitions here -> ACT bias)
                nc.scalar.activation(
                    out=dst[b][:, :], in_=ps[b][:, :], func=AF.Identity,
                    bias=b_sb[:, :], scale=1.0,
                )

        # ---- v natural projection (+ ones column): v_sb[b] [128, 8(sc), 2(h), 65] ----
        v_sb = [vres.tile([128, NI, HPC, HD + 1], f32, tag=f"v{b}", name=f"v_sb{b}") for b in range(B)]
        # PSUM start=True zeroes a whole 2KB bank, so accumulation groups must
        # not share a bank: one sequential group per (b, sc), rotating 1-bank
        # psum tiles. All 8 VT c-tiles stay resident (xin tag has 8 slots).
        v_ts = []
        for ci in range(8):
            v_t = instream.tile([128, B * T], f32, tag="xin", bufs=8, name="v_t")
            nc.sync.dma_start(out=v_t, in_=vt_d[ci * 128 : (ci + 1) * 128, :])
            v_ts.append(v_t)
        for b in range(B):
            for sc in range(NI):
                ps_vt = pp.tile([128, PD], f32, tag="ps_small", bufs=4, name="ps_vt")
                for ci in range(8):
                    nc.tensor.matmul(
                        ps_vt,
                        lhsT=v_ts[ci][:, b * T + sc * 128 : b * T + (sc + 1) * 128],
                        rhs=wv_sb[:, ci, :],
                        start=(ci == 0),
                        stop=(ci == 7),
                    )
                for hl in range(HPC):
                    nc.vector.tensor_add(
                        v_sb[b][:, sc, hl, 0:HD],
                        ps_vt[:, hl * HD : (hl + 1) * HD],
                        bv_sb[:, hl * HD : (hl + 1) * HD],
                    )
            nc.vector.memset(v_sb[b][:, :, :, HD : HD + 1], 1.0)

        # ---- attention per (b, local head) ----
        for b in range(B):
            yt_sb = ytp.tile([PD, T], f32, tag=f"yt{b}")
            for hl in range(HPC):
                p = b * HPC + hl
                qT = qt_sb[b][hl * HD : (hl + 1) * HD, :]
                kT = kt_sb[b][hl * HD : (hl + 1) * HD, :]
                exp_tiles = []
                for i in range(NI):
                    et = expp.tile([128, T], f32, tag="exp")
                    exp_tiles.append(et)
                    jlo = 0 if i < 4 else 1
                    jdiag = 0 if i < 4 else 1
                    for j in range(jlo, NJ):
                        ps = pp.tile([128, 512], f32, tag="ps_small", bufs=4, name="ps_st")
                        nc.tensor.matmul(
                            ps,
                            lhsT=kT[:, i * 128 : (i + 1) * 128],
                            rhs=qT[:, j * 512 : (j + 1) * 512],
                            start=True,
                            stop=True,
                        )
                        sl = slice(j * 512, (j + 1) * 512)
                        if j == jdiag:
                            nc.vector.tensor_add(et[:, sl], ps, ma_sb[:, i, :])
                            nc.scalar.activation(
                                out=et[:, sl], in_=et[:, sl], func=AF.Exp, scale=0.125
                            )
                        else:
                            nc.scalar.activation(
                                out=et[:, sl], in_=ps, func=AF.Exp, scale=0.125
                            )

                for j in range(NJ):
                    ilim = 4 if j == 0 else NI
                    ps_y = pp.tile([HD + 1, 512], f32, tag="ps_small", bufs=4, name="ps_y")
                    for i in range(ilim):
                        nc.tensor.matmul(
                            ps_y,
                            lhsT=v_sb[b][:, i, hl, :],
                            rhs=exp_tiles[i][:, j * 512 : (j + 1) * 512],
                            start=(i == 0),
                            stop=(i == ilim - 1),
                        )
                    rrow = rrow_p.tile([1, 512], f32, tag="rrow")
                    nc.vector.reciprocal(rrow, ps_y[HD : HD + 1, :])
                    # partition-broadcast via DRAM bounce (step-0 partition APs
                    # are only legal on DRAM sources)
                    rd = rdram.tile([1, 512], f32, tag="rd", name="rd")
                    nc.gpsimd.dma_start(out=rd, in_=rrow)
                    rbc = rbc_p.tile([128, 512], f32, tag="rbc")
                    rbc_src = bass.AP(
                        tensor=rd.tensor, offset=rd.offset, ap=[[0, 128], [1, 512]]
                    )
                    nc.gpsimd.dma_start(out=rbc, in_=rbc_src)
                    nc.vector.tensor_mul(
                        yt_sb[hl * HD : (hl + 1) * HD, j * 512 : (j + 1) * 512],
                        ps_y[0:HD, :],
                        rbc[0:HD, :],
                    )
                    for i in range(ilim):
                        s0 = max(i * 128 - j * 512, 0)
                        nc.vector.tensor_mul(
                            exp_tiles[i][:, j * 512 + s0 : (j + 1) * 512],
                            exp_tiles[i][:, j * 512 + s0 : (j + 1) * 512],
                            rbc[:, s0:512],
                        )
                for i in range(NI):
                    nc.scalar.dma_start(
                        out=a_out[p, i * 128 : (i + 1) * 128, i * 128 : T],
                        in_=exp_tiles[i][:, i * 128 : T],
                    )

            # ---- P projection for this batch: y[tc,:] = yT[:,tc]^T @ wp ----
            for tci in range(NI):
                yo = youtp.tile([128, C], f32, tag="yo")
                for co in range(2):
                    ps_p = pp.tile([128, 512], f32, tag="ps_small", bufs=4, name="ps_p")
                    nc.tensor.matmul(
                        ps_p,
                        lhsT=yt_sb[:, tci * 128 : (tci + 1) * 128],
                        rhs=wp_sb[:, co * 512 : (co + 1) * 512],
                        start=True,
                        stop=True,
                    )
                    if co == 0:
                        nc.scalar.activation(
                            out=yo[:, co * 512 : (co + 1) * 512], in_=ps_p,
                            func=AF.Copy, scale=1.0,
                        )
                    else:
                        nc.vector.tensor_copy(yo[:, co * 512 : (co + 1) * 512], ps_p)
                nc.scalar.dma_start(
                    out=y_out[b * T + tci * 128 : b * T + (tci + 1) * 128, :],
                    in_=yo[:, :],
                )

    nc.compile()
    return nc


def _host_prep(inputs):
    Q, K, V = inputs["Q"], inputs["K"], inputs["V"]
    Wq, Wk, Wv, Wp = inputs["Wq"], inputs["Wk"], inputs["Wv"], inputs["Wp"]
    bq, bk, bv = inputs["bq"], inputs["bk"], inputs["bv"]
    mask = inputs["attn_mask"]

    f32 = np.float32
    QT = np.ascontiguousarray(np.asarray(Q, f32).reshape(B * T, C).T)
    KT = np.ascontiguousarray(np.asarray(K, f32).reshape(B * T, C).T)
    VT = np.ascontiguousarray(np.asarray(V, f32).reshape(B * T, C).T)

    # additive mask, transposed to [s, t]; diagonal chunk per s-tile
    ma = np.where(np.asarray(mask).T, f32(-1e30), f32(0.0))
    ma_chunks = np.empty((NI, 128, 512), f32)
    for i in range(NI):
        j = 0 if i < 4 else 1
        ma_chunks[i] = ma[i * 128 : (i + 1) * 128, j * 512 : (j + 1) * 512]
    ma_packed = np.ascontiguousarray(ma_chunks.transpose(1, 0, 2))

    in_maps = []
    for c in range(NCORES):
        hs = slice(c * PD, (c + 1) * PD)

        def packw(W):
            wt = np.asarray(W, f32)[hs].T  # [C, PD]
            return np.ascontiguousarray(wt.reshape(8, 128, PD).transpose(1, 0, 2))

        in_maps.append(
            {
                "qt": QT, "kt": KT, "vt": VT,
                "wqt": packw(Wq), "wkt": packw(Wk), "wvt": packw(Wv),
                "wpt": np.ascontiguousarray(np.asarray(Wp, f32).T[hs]),
                "bq": np.ascontiguousarray(np.asarray(bq, f32)[hs].reshape(PD, 1)),
                "bk": np.ascontiguousarray(np.asarray(bk, f32)[hs].reshape(PD, 1)),
                "bv": np.ascontiguousarray(np.asarray(bv, f32)[hs]),
                "maskadd": ma_packed,
            }
        )
    return in_maps


def _assemble(results, bp):
    # a_core [NP=2b+hl, s, t] -> a[b, 2c+hl, t, s]
    A = np.stack([results[c]["a_out"] for c in range(NCORES)])  # [8, 4, T, T]
    a = np.ascontiguousarray(
        A.reshape(NCORES, B, HPC, T, T).transpose(1, 0, 2, 4, 3)
    ).reshape(B, H, T, T)
    y = np.zeros((B * T, C), np.float32)
    for c in range(NCORES):
        y += results[c]["y_out"]
    y += np.asarray(bp, np.float32)[None, :]
    return y.reshape(B, T, C), a


def kernel(**inputs):
    from concourse.bass_utils import run_bass_kernel_spmd

    if "nc" not in _CACHE:
        _CACHE["nc"] = _build_program()
    nc = _CACHE["nc"]
    in_maps = _host_prep(inputs)
    res = run_bass_kernel_spmd(nc, in_maps, core_ids=list(range(NCORES)))
    y, a = _assemble(res.results, inputs["bp"])
    return y, a


# revision 32
# speedup vs baseline: 572.4454x; 572.4454x over previous
"""Multi-head causal attention (B=2, T=1024, C=1024, H=16) on 8 TRN2 NeuronCores.

Sharding: head/batch parallel. Core c owns heads {2c, 2c+1} for both batches
(4 (b,h) pairs per core). Q/K/V [B,T,C] are host-transposed to [C, B*T] so
every device matmul operand has its contraction dim on SBUF partitions —
no on-device transposes anywhere.

Per-core device pipeline (bf16 operands, fp32 PSUM accumulation):
  1. qT,kT = W @ X^T  -> [128(2 heads x 64), 1024] per batch  (transposed proj)
     v     = X @ W^T  -> [128(s), 2, 65] tiles (natural proj + ones column)
  2. ST = k @ q^T per (b,h): scores in TRANSPOSED layout [s, t]; only the
     causal (lower-triangular) column trapezoid is computed; exp on ACT with
     the 1/8 scale folded in; binary mask multiply on the diagonal block only.
  3. A@V over the live trapezoid with the ones column: one matmul chain
     yields both yT_unnorm = v^T @ exp(ST) and the softmax denominators.
  4. Normalize: reciprocal + gpsimd partition-broadcast; scale yT and the
     exp tiles (-> attention probs, stored transposed as bf16).
  5. y_partial = yT^T @ Wp^T-slice, stored bf16; host sums partials + bp.

Host assembles: a[b, 2c+hl] = a_core[c][2b+hl]^T (upcast), y = sum + bp.
"""

import numpy as np

B, T, C, H = 2, 1024, 1024, 16
HD = C // H          # 64
NCORES = 8
HPC = H // NCORES    # 2 heads per core
PD = HPC * HD        # 128 projected dims per core
NP = B * HPC         # 4 (b,h) pairs per core
NI = T // 128        # 8 s-tiles of 128
NJ = T // 512        # 2 t-chunks of 512

_CACHE = {}


def _build_program():
    import concourse.bass as bass
    import concourse.tile as tile
    import concourse.mybir as mybir
    from concourse import bacc

    f32 = mybir.dt.float32
    bf16 = mybir.dt.bfloat16
    AF = mybir.ActivationFunctionType

    nc = bacc.Bacc("TRN2", target_bir_lowering=False, debug=False, num_devices=NCORES)

    qt_d = nc.dram_tensor("qt", [C, B * T], bf16, kind="ExternalInput")
    kt_d = nc.dram_tensor("kt", [C, B * T], bf16, kind="ExternalInput")
    vt_d = nc.dram_tensor("vt", [C, B * T], bf16, kind="ExternalInput")
    wqt_d = nc.dram_tensor("wqt", [128, 8, PD], bf16, kind="ExternalInput")
    wkt_d = nc.dram_tensor("wkt", [128, 8, PD], bf16, kind="ExternalInput")
    wvt_d = nc.dram_tensor("wvt", [128, 8, PD], bf16, kind="ExternalInput")
    wpt_d = nc.dram_tensor("wpt", [PD, C], bf16, kind="ExternalInput")
    bq_d = nc.dram_tensor("bq", [PD, 1], f32, kind="ExternalInput")
    bk_d = nc.dram_tensor("bk", [PD, 1], f32, kind="ExternalInput")
    bv_d = nc.dram_tensor("bv", [PD], f32, kind="ExternalInput")
    ma_d = nc.dram_tensor("maskmul", [128, NI, 128], bf16, kind="ExternalInput")
    a_out = nc.dram_tensor("a_out", [NP, T, T], bf16, kind="ExternalOutput")
    y_out = nc.dram_tensor("y_out", [B * T, C], bf16, kind="ExternalOutput")

    from contextlib import ExitStack

    with tile.TileContext(nc) as tc, ExitStack() as ctx, nc.allow_low_precision(
        reason="bf16 intermediates are deliberate; fp32 accumulation in PSUM"
    ):
        consts = ctx.enter_context(tc.tile_pool(name="consts", bufs=1))
        instream = ctx.enter_context(tc.tile_pool(name="instream", bufs=4))
        qkres = ctx.enter_context(tc.tile_pool(name="qkres", bufs=1))
        vres = ctx.enter_context(tc.tile_pool(name="vres", bufs=1))
        expp = ctx.enter_context(tc.tile_pool(name="expp", bufs=32))
        rrow_p = ctx.enter_context(tc.tile_pool(name="rrow", bufs=4))
        rbc_p = ctx.enter_context(tc.tile_pool(name="rbc", bufs=4))
        ytp = ctx.enter_context(tc.tile_pool(name="ytp", bufs=1))
        youtp = ctx.enter_context(tc.tile_pool(name="youtp", bufs=3))

        # ---- resident constants ----
        wq_sb = consts.tile([128, 8, PD], bf16)
        nc.sync.dma_start(out=wq_sb, in_=wqt_d[:, :, :])
        wk_sb = consts.tile([128, 8, PD], bf16)
        nc.sync.dma_start(out=wk_sb, in_=wkt_d[:, :, :])
        wv_sb = consts.tile([128, 8, PD], bf16)
        nc.sync.dma_start(out=wv_sb, in_=wvt_d[:, :, :])
        wp_sb = consts.tile([PD, C], bf16)
        nc.sync.dma_start(out=wp_sb, in_=wpt_d[:, :])
        ma_sb = consts.tile([128, NI, 128], bf16)
        nc.sync.dma_start(out=ma_sb, in_=ma_d[:, :, :])
        bq_sb = consts.tile([PD, 1], f32)
        nc.sync.dma_start(out=bq_sb, in_=bq_d[:, :])
        bk_sb = consts.tile([PD, 1], f32)
        nc.sync.dma_start(out=bk_sb, in_=bk_d[:, :])
        # bv broadcast to all partitions: [128, PD]
        bv_sb = consts.tile([128, PD], f32)
        bv_bcast = bass.AP(tensor=bv_d, offset=0, ap=[[0, 128], [1, PD]])
        nc.gpsimd.dma_start(out=bv_sb, in_=bv_bcast)

        pp1_cm = tc.tile_pool(name="pp1", bufs=1, space="PSUM")
        pp1 = pp1_cm.__enter__()

        # ---- VT loads first: v-projection overlaps the q/k streams ----
        v_ts = []
        for ci in range(8):
            v_t = instream.tile([128, B * T], bf16, tag="vin", bufs=8, name="v_t")
            nc.sync.dma_start(out=v_t, in_=vt_d[ci * 128 : (ci + 1) * 128, :])
            v_ts.append(v_t)

        # ---- q/k transposed projections: out[b] = [128(2h*64), 1024(t)] ----
        qt_sb = [qkres.tile([PD, T], bf16, tag=f"qt{b}", name=f"qt_sb{b}") for b in range(B)]
        kt_sb = [qkres.tile([PD, T], bf16, tag=f"kt{b}", name=f"kt_sb{b}") for b in range(B)]
        for name, x_d, w_sb, b_sb, dst in (
            ("q", qt_d, wq_sb, bq_sb, qt_sb),
            ("k", kt_d, wk_sb, bk_sb, kt_sb),
        ):
            ps = [pp1.tile([PD, T], f32, tag="ps_proj", bufs=2, name="ps_proj") for _ in range(B)]
            for ci in range(8):
                x_t = instream.tile([128, B * T], bf16, tag="xin", bufs=4, name="x_t")
                nc.sync.dma_start(out=x_t, in_=x_d[ci * 128 : (ci + 1) * 128, :])
                for b in range(B):
                    for j in range(NJ):
                        nc.tensor.matmul(
                            ps[b][:, j * 512 : (j + 1) * 512],
                            lhsT=w_sb[:, ci, :],
                            rhs=x_t[:, b * T + j * 512 : b * T + (j + 1) * 512],
                            start=(ci == 0),
                            stop=(ci == 7),
                        )
            for b in range(B):
                # evict + bias (bias varies along partitions here -> ACT bias)
                nc.scalar.activation(
                    out=dst[b][:, :], in_=ps[b][:, :], func=AF.Identity,
                    bias=b_sb[:, :], scale=1.0,
                )

        # ---- v natural projection (+ ones column): v_sb[b] [128, 8(sc), 2(h), 65]
        # PSUM start=True zeroes a whole 2KB bank: one sequential accumulation
        # group per (b, sc) on rotating 1-bank psum tiles.
        v_sb = [vres.tile([128, NI, HPC, HD + 1], bf16, tag=f"v{b}", name=f"v_sb{b}") for b in range(B)]
        for b in range(B):
            for sc in range(NI):
                ps_vt = pp1.tile([128, PD], f32, tag="ps_vt", bufs=4, name="ps_vt")
                for ci in range(8):
                    nc.tensor.matmul(
                        ps_vt,
                        lhsT=v_ts[ci][:, b * T + sc * 128 : b * T + (sc + 1) * 128],
                        rhs=wv_sb[:, ci, :],
                        start=(ci == 0),
                        stop=(ci == 7),
                    )
                nc.vector.tensor_add(
                    v_sb[b][:, sc, :, 0:HD],
                    ps_vt[:, :].rearrange("p (h d) -> p h d", h=HPC),
                    bv_sb[:, :].rearrange("p (h d) -> p h d", h=HPC),
                )
            nc.vector.memset(v_sb[b][:, :, :, HD : HD + 1], 1.0)

        pp1_cm.__exit__(None, None, None)
        pp2 = ctx.enter_context(tc.tile_pool(name="pp2", bufs=1, space="PSUM"))

        # ---- attention, pair-skewed emission so per-engine FIFOs pipeline ----
        all_exp = {}

        def emit_scores(b, hl):
            qT = qt_sb[b][hl * HD : (hl + 1) * HD, :]
            kT = kt_sb[b][hl * HD : (hl + 1) * HD, :]
            exp_tiles = []
            for i in range(NI):
                et = expp.tile([128, T], bf16, tag="exp", name="et")
                exp_tiles.append(et)
                jlo = 0 if i < 4 else 1
                for j in range(jlo, NJ):
                    lo = max(i * 128, j * 512)
                    hi = (j + 1) * 512
                    ps = pp2.tile([128, 512], f32, tag="ps_st", bufs=4, name="ps_st")
                    nc.tensor.matmul(
                        ps[:, 0 : hi - lo],
                        lhsT=kT[:, i * 128 : (i + 1) * 128],
                        rhs=qT[:, lo:hi],
                        start=True,
                        stop=True,
                    )
                    nc.scalar.activation(
                        out=et[:, lo:hi], in_=ps[:, 0 : hi - lo], func=AF.Exp,
                        scale=0.125,
                    )
                # zero the masked upper-triangular part of the diagonal
                # 128x128 block (binary bf16 mask)
                nc.vector.tensor_mul(
                    et[:, i * 128 : (i + 1) * 128],
                    et[:, i * 128 : (i + 1) * 128],
                    ma_sb[:, i, :],
                )
            all_exp[(b, hl)] = exp_tiles

        yt_tiles = {}

        def emit_av(b, hl):
            if b not in yt_tiles:
                yt_tiles[b] = ytp.tile([PD, T], bf16, tag=f"yt{b}", name=f"yt_sb{b}")
            yt_sb = yt_tiles[b]
            p = b * HPC + hl
            exp_tiles = all_exp[(b, hl)]
            for j in range(NJ):
                ilim = 4 if j == 0 else NI
                ps_y = pp2.tile([HD + 1, 512], f32, tag="ps_y", bufs=2, name="ps_y")
                for i in range(ilim):
                    s0 = max(i * 128 - j * 512, 0)
                    nc.tensor.matmul(
                        ps_y[:, s0:512],
                        lhsT=v_sb[b][:, i, hl, :],
                        rhs=exp_tiles[i][:, j * 512 + s0 : (j + 1) * 512],
                        start=(i == 0),
                        stop=(i == ilim - 1),
                    )
                rrow = rrow_p.tile([1, 512], bf16, tag="rrow", name="rrow")
                nc.vector.reciprocal(rrow, ps_y[HD : HD + 1, :])
                rbc = rbc_p.tile([128, 512], bf16, tag="rbc", name="rbc")
                nc.gpsimd.partition_broadcast(rbc, rrow)
                nc.vector.tensor_mul(
                    yt_sb[hl * HD : (hl + 1) * HD, j * 512 : (j + 1) * 512],
                    ps_y[0:HD, :],
                    rbc[0:HD, :],
                )
                for i in range(ilim):
                    s0 = max(i * 128 - j * 512, 0)
                    nc.vector.tensor_mul(
                        exp_tiles[i][:, j * 512 + s0 : (j + 1) * 512],
                        exp_tiles[i][:, j * 512 + s0 : (j + 1) * 512],
                        rbc[:, s0:512],
                    )
            for i in range(NI):
                nc.sync.dma_start(
                    out=a_out[p, i * 128 : (i + 1) * 128, i * 128 : T],
                    in_=exp_tiles[i][:, i * 128 : T],
                )

        def emit_p(b):
            yt_sb = yt_tiles[b]
            for tci in range(NI):
                yo = youtp.tile([128, C], bf16, tag="yo", name="yo")
                for co in range(2):
                    ps_p = pp2.tile([128, 512], f32, tag="ps_p", bufs=2, name="ps_p")
                    nc.tensor.matmul(
                        ps_p,
                        lhsT=yt_sb[:, tci * 128 : (tci + 1) * 128],
                        rhs=wp_sb[:, co * 512 : (co + 1) * 512],
                        start=True,
                        stop=True,
                    )
                    if co == 0:
                        nc.scalar.activation(
                            out=yo[:, co * 512 : (co + 1) * 512], in_=ps_p,
                            func=AF.Copy, scale=1.0,
                        )
                    else:
                        nc.vector.tensor_copy(yo[:, co * 512 : (co + 1) * 512], ps_p)
                nc.sync.dma_start(
                    out=y_out[b * T + tci * 128 : b * T + (tci + 1) * 128, :],
                    in_=yo[:, :],
                )

        # skewed schedule: scores one pair ahead of A@V
        pairs = [(b, hl) for b in range(B) for hl in range(HPC)]
        emit_scores(*pairs[0])
        emit_scores(*pairs[1])
        emit_av(*pairs[0])
        emit_scores(*pairs[2])
        emit_av(*pairs[1])
        emit_p(0)
        emit_scores(*pairs[3])
        emit_av(*pairs[2])
        emit_av(*pairs[3])
        emit_p(1)

    nc.compile()
    return nc


def _host_prep(inputs):
    Q, K, V = inputs["Q"], inputs["K"], inputs["V"]
    Wq, Wk, Wv, Wp = inputs["Wq"], inputs["Wk"], inputs["Wv"], inputs["Wp"]
    bq, bk, bv = inputs["bq"], inputs["bk"], inputs["bv"]
    mask = inputs["attn_mask"]

    import ml_dtypes

    f32 = np.float32
    bf16 = ml_dtypes.bfloat16
    QT = np.ascontiguousarray(np.asarray(Q, f32).reshape(B * T, C).T.astype(bf16))
    KT = np.ascontiguousarray(np.asarray(K, f32).reshape(B * T, C).T.astype(bf16))
    VT = np.ascontiguousarray(np.asarray(V, f32).reshape(B * T, C).T.astype(bf16))

    # binary keep-mask (1=keep, 0=masked) for the diagonal 128x128 block of
    # each s-tile, transposed to [s, t]
    ma = np.where(np.asarray(mask).T, f32(0.0), f32(1.0)).astype(bf16)
    ma_chunks = np.empty((NI, 128, 128), bf16)
    for i in range(NI):
        ma_chunks[i] = ma[i * 128 : (i + 1) * 128, i * 128 : (i + 1) * 128]
    ma_packed = np.ascontiguousarray(ma_chunks.transpose(1, 0, 2))

    in_maps = []
    for c in range(NCORES):
        hs = slice(c * PD, (c + 1) * PD)

        def packw(W):
            wt = np.asarray(W, f32)[hs].T.astype(bf16)  # [C, PD]
            return np.ascontiguousarray(wt.reshape(8, 128, PD).transpose(1, 0, 2))

        in_maps.append(
            {
                "qt": QT, "kt": KT, "vt": VT,
                "wqt": packw(Wq), "wkt": packw(Wk), "wvt": packw(Wv),
                "wpt": np.ascontiguousarray(np.asarray(Wp, f32).T[hs].astype(bf16)),
                "bq": np.ascontiguousarray(np.asarray(bq, f32)[hs].reshape(PD, 1)),
                "bk": np.ascontiguousarray(np.asarray(bk, f32)[hs].reshape(PD, 1)),
                "bv": np.ascontiguousarray(np.asarray(bv, f32)[hs]),
                "maskmul": ma_packed,
            }
        )
    return in_maps


def _assemble(results, bp):
    # a_core [NP=2b+hl, s, t] -> a[b, 2c+hl, t, s]
    A = np.stack(
        [np.asarray(results[c]["a_out"], np.float32) for c in range(NCORES)]
    )  # [8, 4, T, T]
    a = np.ascontiguousarray(
        A.reshape(NCORES, B, HPC, T, T).transpose(1, 0, 2, 4, 3)
    ).reshape(B, H, T, T)
    y = np.zeros((B * T, C), np.float32)
    for c in range(NCORES):
        y += np.asarray(results[c]["y_out"], np.float32)
    y += np.asarray(bp, np.float32)[None, :]
    return y.reshape(B, T, C), a


def kernel(**inputs):
    from concourse.bass_utils import run_bass_kernel_spmd

    if "nc" not in _CACHE:
        _CACHE["nc"] = _build_program()
    nc = _CACHE["nc"]
    in_maps = _host_prep(inputs)
    try:
        res = run_bass_kernel_spmd(nc, in_maps, core_ids=list(range(NCORES)))
    except Exception:
        # one retry: a previous crashed session can leave the device wedged
        res = run_bass_kernel_spmd(nc, in_maps, core_ids=list(range(NCORES)))
    y, a = _assemble(res.results, inputs["bp"])
    return y, a
